# revision 18
# baseline (speedup 1.0000x reference)
"""DynamicLinear (MoE routing) Trainium2 Bass kernel.

Math (per sample b):
    out[b] = sum_k attn[b,k] * (x[b] @ W[k].T + bias[k])
           = sum_k attn[b,k] * (x[b] @ W[k].T) + attn[b] @ bias

Sharding: 8 cores in a 2x4 grid over (batch, out_features).
Each core computes out[b_half, o_quarter] from x[b_half] (16 MiB fp32)
and W[:, o_quarter, :] (16 MiB fp32) -- no cross-core communication.

The host ships x and W pre-transposed (i-major: xT [IN, BL] and
wT [K, IN, OL], still fp32) so the device needs no casts and no
transposes: every SBUF load is a plain strided HWDGE DMA that puts the
contraction dim on partitions, and the matmuls run as float32r (full
PE rate at N=512, fp32 storage, reduced-precision multiplies).

Per-core schedule (expert pairs keep SBUF small; x is streamed twice):
  phase 0: experts {0,1}  x  b_tiles 0..15
  phase 1: experts {2,3}  x  b_tiles 0..15
Per (b_tile, expert): 16 matmul passes (K=128 contraction, N=512
moving) accumulate in one PSUM bank; DVE combines
acc[t] = sum_k attn[:,k]*(bias[k] + psum_k) with attn as per-partition
scalar; out stores after the last expert.
"""

import numpy as np

_B, _K, _IN, _OUT = 4096, 4, 2048, 2048
_GRID_B, _GRID_O = 2, 4
_BL = _B // _GRID_B      # 2048 batch rows per core
_OL = _OUT // _GRID_O    # 512 out cols per core
_NBT = _BL // 128        # 16 b tiles
_NIT = _IN // 128        # 16 contraction tiles

_CACHE = {}
LAST_RESULTS = None


def _build_program():
    import concourse.bass as bass
    import concourse.tile as tile
    from concourse import bacc, mybir

    f32 = mybir.dt.float32
    f32r = mybir.dt.float32r
    MULT = mybir.AluOpType.mult
    ADD = mybir.AluOpType.add

    nc = bacc.Bacc("TRN2", target_bir_lowering=False, debug=False)
    # host-pretiled layouts: every load is contiguous per partition
    bf16 = mybir.dt.bfloat16
    xT = nc.dram_tensor("xT", [_NBT, 128, _NIT, 128], bf16,
                        kind="ExternalInput").ap()
    attn = nc.dram_tensor("attn", [_BL, _K], f32, kind="ExternalInput").ap()
    wT = nc.dram_tensor("wT", [_K, 128, _NIT, _OL], bf16,
                        kind="ExternalInput").ap()
    bias = nc.dram_tensor("bias", [_K, _OL], f32, kind="ExternalInput").ap()
    out = nc.dram_tensor("out", [_BL, _OL], f32, kind="ExternalOutput").ap()

    _NH = _NIT // 4  # ii-tiles per W load granule

    with tile.TileContext(nc) as tc:
        with (
            tc.tile_pool(name="wt", bufs=4 * _K) as wtp,
            tc.tile_pool(name="xt", bufs=_NBT) as xtp,
            tc.tile_pool(name="singles", bufs=1) as singles,
            tc.tile_pool(name="acc", bufs=_NBT) as accp,
            tc.tile_pool(name="psum", bufs=8, space="PSUM") as psump,
        ):
            def load_w_part(k, h):
                # wt[k][h][i_in, j, o] = W[k][o, (h*NH + j)*128 + i_in]
                # 0.5 MiB granules so the HBM ramp feeds the PE just-in-time
                t_ = wtp.tile([128, _NH, _OL], bf16, tag="wt",
                              name=f"wt{k}_{h}")
                nc.sync.dma_start(out=t_, in_=wT[k, :, h * _NH:(h + 1) * _NH])
                return t_

            def load_x(t):
                # xt[t][i_in, ii, b] = x[t*128 + b, ii*128 + i_in]
                t_ = xtp.tile([128, _NIT, 128], bf16, tag="xt",
                              name=f"xt{t}")
                nc.scalar.dma_start(out=t_, in_=xT[t])
                return t_

            # bias replicated across all 128 partitions (SWDGE, small)
            bias_rep = singles.tile([128, _K, _OL], f32)
            nc.gpsimd.dma_start(
                out=bias_rep,
                in_=bass.AP(
                    tensor=bias.tensor,
                    offset=bias.offset,
                    ap=[[0, 128], bias.ap[0], bias.ap[1]],
                ),
            )

            # W granules on sync in need-order; x stream on scalar.
            wt = {(0, 0): load_w_part(0, 0)}
            xts = {0: load_x(0)}
            for h in range(1, _NIT // _NH):
                wt[(0, h)] = load_w_part(0, h)
            for h in range(_NIT // _NH):
                wt[(1, h)] = load_w_part(1, h)

            # attn for all b_tiles, b on partitions (sync ring, after the
            # phase-0 weights): attn_sb[p, t, k] = attn[t*128 + p, k]
            attn_sb = singles.tile([128, _NBT, _K], f32)
            attn_src = bass.AP(
                tensor=attn.tensor,
                offset=attn.offset,
                ap=[[_K, 128], [128 * _K, _NBT], [1, _K]],
            )
            nc.sync.dma_start(out=attn_sb, in_=attn_src)

            for k in (2, 3):
                for h in range(_NIT // _NH):
                    wt[(k, h)] = load_w_part(k, h)
            acc = [None] * _NBT

            for phase in range(2):
                ks = (2 * phase, 2 * phase + 1)
                for t in range(_NBT):
                    if t not in xts:
                        xts[t] = load_x(t)
                    xt = xts[t]
                    a_sc = attn_sb[:, t, :]
                    for k in ks:
                        ps = psump.tile([128, _OL], f32, tag="ps",
                                        name=f"ps{k}_{t}")
                        for ii in range(_NIT):
                            nc.tensor.matmul(
                                ps,
                                lhsT=xt[:, ii, :],
                                rhs=wt[(k, ii // _NH)][:, ii % _NH, :],
                                start=(ii == 0), stop=(ii == _NIT - 1),
                            )
                        if k == 0:
                            # init acc with the full bias combination (DVE)
                            at = accp.tile([128, _OL], f32, tag="acc",
                                           name=f"acc{t}")
                            acc[t] = at
                            nc.vector.tensor_scalar(
                                out=at, in0=bias_rep[:, 0, :],
                                scalar1=a_sc[:, 0:1], scalar2=None, op0=MULT,
                            )
                            for kk in range(1, _K):
                                nc.vector.scalar_tensor_tensor(
                                    out=at, in0=bias_rep[:, kk, :],
                                    scalar=a_sc[:, kk:kk + 1], in1=at,
                                    op0=MULT, op1=ADD,
                                )
                        nc.vector.scalar_tensor_tensor(
                            out=acc[t], in0=ps, scalar=a_sc[:, k:k + 1],
                            in1=acc[t], op0=MULT, op1=ADD,
                        )
                        if k == _K - 1:
                            nc.sync.dma_start(
                                out=out[t * 128:(t + 1) * 128, :],
                                in_=acc[t],
                            )

    nc.compile()
    return nc


def _get_program():
    if "nc" not in _CACHE:
        _CACHE["nc"] = _build_program()
    return _CACHE["nc"]


def _ensure_axon_hooks_importable():
    """bass_utils' trace branch imports antenv.axon_hooks, which the
    trimmed agent image may lack; stub it (hook=None) so a stray
    BASS_TRACE=1 degrades to an untraced run instead of crashing."""
    import sys
    import types

    try:
        import antenv.axon_hooks  # noqa: F401
        return
    except ImportError:
        pass
    mod = types.ModuleType("antenv.axon_hooks")
    mod._hook = None
    mod.get_axon_ntff_profile_hook = lambda: mod._hook

    def _set(h):
        mod._hook = h

    mod.set_axon_ntff_profile_hook = _set
    sys.modules["antenv.axon_hooks"] = mod
    try:
        import antenv
        antenv.axon_hooks = mod
    except ImportError:
        pass


def kernel(**inputs):
    global LAST_RESULTS
    from concourse.bass_utils import run_bass_kernel_spmd

    _ensure_axon_hooks_importable()

    x = np.ascontiguousarray(inputs["x"], dtype=np.float32)
    attn = np.ascontiguousarray(inputs["softmax_attention"], dtype=np.float32)
    w = np.ascontiguousarray(inputs["weight"], dtype=np.float32)
    b = np.ascontiguousarray(inputs["bias"], dtype=np.float32)

    nc = _get_program()
    in_maps = []
    for c in range(8):
        gb, go = divmod(c, _GRID_O)
        x_sl = x[gb * _BL:(gb + 1) * _BL]
        w_sl = w[:, go * _OL:(go + 1) * _OL, :]
        # tile-contiguous device layouts (see _build_program):
        # xT[t, i_in, ii, b_in] = x[t*128 + b_in, ii*128 + i_in]
        # wT[k, i_in, ii, o]    = W[k, o, ii*128 + i_in]
        import ml_dtypes
        xT = np.ascontiguousarray(
            x_sl.T.reshape(_NIT, 128, _NBT, 128).transpose(2, 1, 0, 3)
        ).astype(ml_dtypes.bfloat16)
        wTa = np.ascontiguousarray(
            w_sl.transpose(0, 2, 1)
            .reshape(_K, _NIT, 128, _OL).transpose(0, 2, 1, 3)
        ).astype(ml_dtypes.bfloat16)
        in_maps.append({
            "xT": xT,
            "attn": np.ascontiguousarray(attn[gb * _BL:(gb + 1) * _BL]),
            "wT": wTa,
            "bias": np.ascontiguousarray(b[:, go * _OL:(go + 1) * _OL]),
        })

    res = run_bass_kernel_spmd(nc, in_maps, list(range(8)))
    LAST_RESULTS = res

    full = np.empty((_B, _OUT), dtype=np.float32)
    for c in range(8):
        gb, go = divmod(c, _GRID_O)
        full[gb * _BL:(gb + 1) * _BL, go * _OL:(go + 1) * _OL] = \
            res.results[c]["out"]
    return full


# revision 19
# speedup vs baseline: 1.0206x; 1.0206x over previous
"""DynamicLinear (MoE routing) Trainium2 Bass kernel.

Math (per sample b):
    out[b] = sum_k attn[b,k] * (x[b] @ W[k].T + bias[k])
           = sum_k attn[b,k] * (x[b] @ W[k].T) + attn[b] @ bias

Sharding: 8 cores in a 2x4 grid over (batch, out_features).
Each core computes out[b_half, o_quarter] from x[b_half] (16 MiB fp32)
and W[:, o_quarter, :] (16 MiB fp32) -- no cross-core communication.

The host ships x and W pre-transposed (i-major: xT [IN, BL] and
wT [K, IN, OL], still fp32) so the device needs no casts and no
transposes: every SBUF load is a plain strided HWDGE DMA that puts the
contraction dim on partitions, and the matmuls run as float32r (full
PE rate at N=512, fp32 storage, reduced-precision multiplies).

Per-core schedule (expert pairs keep SBUF small; x is streamed twice):
  phase 0: experts {0,1}  x  b_tiles 0..15
  phase 1: experts {2,3}  x  b_tiles 0..15
Per (b_tile, expert): 16 matmul passes (K=128 contraction, N=512
moving) accumulate in one PSUM bank; DVE combines
acc[t] = sum_k attn[:,k]*(bias[k] + psum_k) with attn as per-partition
scalar; out stores after the last expert.
"""

import numpy as np

_B, _K, _IN, _OUT = 4096, 4, 2048, 2048
_GRID_B, _GRID_O = 2, 4
_BL = _B // _GRID_B      # 2048 batch rows per core
_OL = _OUT // _GRID_O    # 512 out cols per core
_NBT = _BL // 128        # 16 b tiles
_NIT = _IN // 128        # 16 contraction tiles

_CACHE = {}
LAST_RESULTS = None


def _build_program():
    import concourse.bass as bass
    import concourse.tile as tile
    from concourse import bacc, mybir

    f32 = mybir.dt.float32
    f32r = mybir.dt.float32r
    MULT = mybir.AluOpType.mult
    ADD = mybir.AluOpType.add

    nc = bacc.Bacc("TRN2", target_bir_lowering=False, debug=False)
    # host-pretiled layouts: every load is contiguous per partition
    bf16 = mybir.dt.bfloat16
    xT = nc.dram_tensor("xT", [_NBT, 128, _NIT, 128], bf16,
                        kind="ExternalInput").ap()
    attn = nc.dram_tensor("attn", [_BL, _K], f32, kind="ExternalInput").ap()
    wT = nc.dram_tensor("wT", [_K, 128, _NIT, _OL], bf16,
                        kind="ExternalInput").ap()
    bias = nc.dram_tensor("bias", [_K, _OL], f32, kind="ExternalInput").ap()
    out = nc.dram_tensor("out", [_BL, _OL], f32, kind="ExternalOutput").ap()

    _NH = _NIT // 4  # ii-tiles per W load granule

    with tile.TileContext(nc) as tc:
        with (
            tc.tile_pool(name="wt", bufs=4 * _K) as wtp,
            tc.tile_pool(name="xt", bufs=_NBT) as xtp,
            tc.tile_pool(name="singles", bufs=1) as singles,
            tc.tile_pool(name="acc", bufs=_NBT) as accp,
            tc.tile_pool(name="psum", bufs=8, space="PSUM") as psump,
        ):
            def load_w_part(k, h):
                # wt[k][h][i_in, j, o] = W[k][o, (h*NH + j)*128 + i_in]
                # 0.5 MiB granules so the HBM ramp feeds the PE just-in-time
                t_ = wtp.tile([128, _NH, _OL], bf16, tag="wt",
                              name=f"wt{k}_{h}")
                nc.sync.dma_start(out=t_, in_=wT[k, :, h * _NH:(h + 1) * _NH])
                return t_

            def load_x(t):
                # xt[t][i_in, ii, b] = x[t*128 + b, ii*128 + i_in]
                t_ = xtp.tile([128, _NIT, 128], bf16, tag="xt",
                              name=f"xt{t}")
                nc.scalar.dma_start(out=t_, in_=xT[t])
                return t_

            # bias replicated across all 128 partitions (SWDGE, small)
            bias_rep = singles.tile([128, _K, _OL], f32)
            nc.gpsimd.dma_start(
                out=bias_rep,
                in_=bass.AP(
                    tensor=bias.tensor,
                    offset=bias.offset,
                    ap=[[0, 128], bias.ap[0], bias.ap[1]],
                ),
            )

            # W granules on sync in need-order; x stream on scalar.
            wt = {(0, 0): load_w_part(0, 0)}
            xts = {0: load_x(0)}
            for h in range(1, _NIT // _NH):
                wt[(0, h)] = load_w_part(0, h)
            for h in range(_NIT // _NH):
                wt[(1, h)] = load_w_part(1, h)

            # attn for all b_tiles, b on partitions (sync ring, after the
            # phase-0 weights): attn_sb[p, t, k] = attn[t*128 + p, k]
            attn_sb = singles.tile([128, _NBT, _K], f32)
            attn_src = bass.AP(
                tensor=attn.tensor,
                offset=attn.offset,
                ap=[[_K, 128], [128 * _K, _NBT], [1, _K]],
            )
            nc.sync.dma_start(out=attn_sb, in_=attn_src)

            for k in (2, 3):
                for h in range(_NIT // _NH):
                    wt[(k, h)] = load_w_part(k, h)
            acc = [None] * _NBT

            for k in range(_K):
                for t in range(_NBT):
                    if t not in xts:
                        xts[t] = load_x(t)
                    xt = xts[t]
                    a_sc = attn_sb[:, t, :]
                    if True:
                        ps = psump.tile([128, _OL], f32, tag="ps",
                                        name=f"ps{k}_{t}")
                        for ii in range(_NIT):
                            nc.tensor.matmul(
                                ps,
                                lhsT=xt[:, ii, :],
                                rhs=wt[(k, ii // _NH)][:, ii % _NH, :],
                                start=(ii == 0), stop=(ii == _NIT - 1),
                            )
                        if k == 0:
                            # init acc with the full bias combination (DVE)
                            at = accp.tile([128, _OL], f32, tag="acc",
                                           name=f"acc{t}")
                            acc[t] = at
                            nc.vector.tensor_scalar(
                                out=at, in0=bias_rep[:, 0, :],
                                scalar1=a_sc[:, 0:1], scalar2=None, op0=MULT,
                            )
                            for kk in range(1, _K):
                                nc.vector.scalar_tensor_tensor(
                                    out=at, in0=bias_rep[:, kk, :],
                                    scalar=a_sc[:, kk:kk + 1], in1=at,
                                    op0=MULT, op1=ADD,
                                )
                        nc.vector.scalar_tensor_tensor(
                            out=acc[t], in0=ps, scalar=a_sc[:, k:k + 1],
                            in1=acc[t], op0=MULT, op1=ADD,
                        )
                        if k == _K - 1:
                            nc.sync.dma_start(
                                out=out[t * 128:(t + 1) * 128, :],
                                in_=acc[t],
                            )

    nc.compile()
    return nc


def _get_program():
    if "nc" not in _CACHE:
        _CACHE["nc"] = _build_program()
    return _CACHE["nc"]


def _ensure_axon_hooks_importable():
    """bass_utils' trace branch imports antenv.axon_hooks, which the
    trimmed agent image may lack; stub it (hook=None) so a stray
    BASS_TRACE=1 degrades to an untraced run instead of crashing."""
    import sys
    import types

    try:
        import antenv.axon_hooks  # noqa: F401
        return
    except ImportError:
        pass
    mod = types.ModuleType("antenv.axon_hooks")
    mod._hook = None
    mod.get_axon_ntff_profile_hook = lambda: mod._hook

    def _set(h):
        mod._hook = h

    mod.set_axon_ntff_profile_hook = _set
    sys.modules["antenv.axon_hooks"] = mod
    try:
        import antenv
        antenv.axon_hooks = mod
    except ImportError:
        pass


def kernel(**inputs):
    global LAST_RESULTS
    from concourse.bass_utils import run_bass_kernel_spmd

    _ensure_axon_hooks_importable()

    x = np.ascontiguousarray(inputs["x"], dtype=np.float32)
    attn = np.ascontiguousarray(inputs["softmax_attention"], dtype=np.float32)
    w = np.ascontiguousarray(inputs["weight"], dtype=np.float32)
    b = np.ascontiguousarray(inputs["bias"], dtype=np.float32)

    nc = _get_program()
    in_maps = []
    for c in range(8):
        gb, go = divmod(c, _GRID_O)
        x_sl = x[gb * _BL:(gb + 1) * _BL]
        w_sl = w[:, go * _OL:(go + 1) * _OL, :]
        # tile-contiguous device layouts (see _build_program):
        # xT[t, i_in, ii, b_in] = x[t*128 + b_in, ii*128 + i_in]
        # wT[k, i_in, ii, o]    = W[k, o, ii*128 + i_in]
        import ml_dtypes
        xT = np.ascontiguousarray(
            x_sl.T.reshape(_NIT, 128, _NBT, 128).transpose(2, 1, 0, 3)
        ).astype(ml_dtypes.bfloat16)
        wTa = np.ascontiguousarray(
            w_sl.transpose(0, 2, 1)
            .reshape(_K, _NIT, 128, _OL).transpose(0, 2, 1, 3)
        ).astype(ml_dtypes.bfloat16)
        in_maps.append({
            "xT": xT,
            "attn": np.ascontiguousarray(attn[gb * _BL:(gb + 1) * _BL]),
            "wT": wTa,
            "bias": np.ascontiguousarray(b[:, go * _OL:(go + 1) * _OL]),
        })

    res = run_bass_kernel_spmd(nc, in_maps, list(range(8)))
    LAST_RESULTS = res

    full = np.empty((_B, _OUT), dtype=np.float32)
    for c in range(8):
        gb, go = divmod(c, _GRID_O)
        full[gb * _BL:(gb + 1) * _BL, go * _OL:(go + 1) * _OL] = \
            res.results[c]["out"]
    return full
